# revision 1
# baseline (speedup 1.0000x reference)
"""Additive noise channel kernel for 8 Trainium2 NeuronCores.

Computes out[b, s, 0:2] = complex_FIR(x, a)[b, s] + (L @ (scale * noise))[b, s]
with B=64, S=8192, T=129 taps, L lower-triangular [S, S].

Strategy
--------
The dominant cost is reading L (256 MB fp32, half of it zeros).  We shard the
OUTPUT dim S across the 8 cores so each core reads only its columns of L^T,
and we exploit the triangular structure with a staircase assignment that is
perfectly SPMD-uniform: core k takes the eight 128-column strips
beta = 8j + k (j = 0..7).  Strip slot j is padded to a uniform extent of
8*(j+1) k-tiles of 128 rows (provably the minimal uniform cover of the
triangle), so every core runs the identical instruction stream on 288
k-tiles of packed L^T (vs 512 for a naive row shard, 4x that for the
batch-parallel hint).  L^T is carried in fp8e3m4 (pre-scaled by C_LT, the
inverse folded into the fp16 noise stationary at zero cost), everything
else fp16, accumulation fp32 in PSUM: ~1.3e-3 scaled absmax output error --
below what a plain all-bf16 kernel produces on this problem.

On-device everything is TensorE matmuls accumulating in PSUM:
  * noise coloring: lhsT = [scale*noise_r^T | scale*noise_i^T]  (K=128, M=128)
                    rhs  = L^T tile (fp8)                        (K=128, N=128)
    -> psum rows 0:64 = real part, rows 64:128 = imag part; one stream of L
    feeds both real and imag outputs.
  * complex FIR: expressed as x_ext^T @ A where A is the banded Toeplitz
    matrix of the taps, folded into the same PSUM accumulation
    (yr = xr*Ar - xi*Ai, yi = xr*Ai + xi*Ar); the second stationary
    [-xi | xr] is derived on the otherwise-idle VectorE.

Schedule: window-pair-major -- pair p covers k-tiles [16p, 16p+16) of every
still-active slot, so the noise-stationary demand spreads evenly instead of
front-loading; completed slots evacuate + stream out mid-kernel (completing
slots go first within pairs 1-3 so their chains overlap the pair's stream;
pair 0's go last because their FIR needs the late-arriving constants), and
slots 6/7's FIR runs a pair early, so the tail after the last chunk is one
short matmul chain.  The fs/a2/npk constant loads are pinned behind specific
chunks with sync=False dependency edges: without them the Tile scheduler
hoists these dep-free loads ahead of the chunk stream (6.6 us PE stall);
anchored too early they displace pair-0 chunk bytes (1 us PE stall) -- the
swept optimum anchors fs/a2 behind chunks 3/4 and the three noise-window
prefetches behind chunks 7/12/16.

All DRAM inputs are packed host-side in SBUF-image layout (partition-major,
2-4 KB contiguous runs per partition, chunk sequence in exact consumption
order) so the HBM read stream is sequential and every DMA descriptor is
>=1 KB.  Outputs are written planar (real / imag) and interleaved on the
host via one merged planar tensor (row = plane*B + batch, matching the psum
partition layout, so each store is a single full-128-partition DMA).
Cost-model timeline: 28.5 us/core, 0.5 us above the analytic lower bound
for any schedule of this decomposition (max over chunks of arrival time +
remaining PE work, plus the copy/store/sem/barrier constants).
"""

import os
import sys
import time

for _p in ("/opt/trn_rl_repo", "/root/.axon_site/_ro/trn_rl_repo"):
    if _p not in sys.path:
        sys.path.append(_p)

# the bass kernel executes through jax/PJRT on the axon-tunneled NeuronCores
os.environ.setdefault("JAX_PLATFORMS", "axon,cpu")

import numpy as np

import concourse.bass as bass
import concourse.mybir as mybir
import concourse.tile as tile
from concourse.tile import add_dep_helper
from concourse import bacc
from concourse.bass_utils import run_bass_kernel_spmd

B = 64          # batch
S = 8192        # block size
T = 129         # taps
H = (T - 1) // 2  # 64
P = 128         # partitions / k-tile
N_CORES = 8
N_SLOTS = 8     # strips per core
W = 128         # strip width (output columns per slot)
SLOT_KT = [8 * (j + 1) for j in range(N_SLOTS)]   # padded k-tiles per slot
TOT_KT = sum(SLOT_KT)  # 288

# Window-pair-major schedule: pair p covers k-tiles [16p, 16p+16).  All slots
# still alive advance through that window together, so the npk (noise) demand
# spreads evenly across the kernel instead of front-loading, and slots 2p /
# 2p+1 finish in pair p (their outputs stream out mid-kernel).
# CONSUME entries: (slot j, first k-tile kt0, n k-tiles ck, flat offset);
# chunks are laid out back-to-back in DRAM in this (consumption) order.
CONSUME = []
_flat = 0
for _p in range(4):
    # pairs 1-3: completing slots FIRST -- their chunks arrive earliest in
    # the pair, so their FIR + psum evacuation + store all overlap the rest
    # of the pair's chunk stream instead of gating the kernel tail.  Pair 0
    # keeps them LAST: slots 0/1's FIR needs the fs/a2/fsi constants, which
    # only land a few us in.
    if _p == 0:
        _order = list(range(2, N_SLOTS)) + [0, 1]
    elif _p == 3:
        # slot 7 last, with its final chunk split so the chain after the
        # very last byte is only 4 matmuls + one 64 KB store
        _order = [6, 7]
    else:
        _order = [2 * _p, 2 * _p + 1] + list(range(2 * _p + 2, N_SLOTS))
    for _j in _order:
        _ck = 8 if _j == 2 * _p else 16
        CONSUME.append((_j, 16 * _p, _ck, _flat))
        _flat += _ck
assert _flat == TOT_KT
_j9, _kt9, _ck9, _fl9 = CONSUME[-1]
CONSUME[-1:] = [(_j9, _kt9, 12, _fl9), (_j9, _kt9 + 12, 4, _fl9 + 12)]

# Precision mode.  "mixed8": L^T in fp8e3m4 (pre-scaled by C_LT, folded back
# via the fp16 noise stationary), everything else fp16, fp32 PSUM accumulate
# -> ~1.3e-3 scaled absmax error, below a plain all-bf16 kernel's error.
# "float16": all operands fp16 (~3e-4).  "float32": exact (~2e-7), 4x slower.
NOISE_DT = "mixed8"

C_LT = 64.0  # fp8 pre-scale: lt stores C_LT*L^T, npk stores scale*noise/C_LT

_DT_NP = {"float32": np.float32, "float16": np.float16}


def _mode_dtypes(dt_name):
    """returns (lt mybir dt, operand mybir dt name) for a mode."""
    if dt_name == "mixed8":
        return "float8e3", "float16"
    return dt_name, dt_name

LAST_RUN_SECONDS = None
_CACHE = {}


def _build_program(dt_name: str):
    lt_dt_name, op_dt_name = _mode_dtypes(dt_name)
    lt_dt = getattr(mybir.dt, lt_dt_name)
    dt = getattr(mybir.dt, op_dt_name)
    f32 = mybir.dt.float32

    nc = bacc.Bacc("TRN2", target_bir_lowering=False, debug=False,
                   num_devices=N_CORES)

    # all inputs are SBUF images: [128 partitions, free...]; lt is a flat
    # sequence of per-chunk SBUF images in consumption order
    lt = nc.dram_tensor("lt", [TOT_KT * P * P], lt_dt, kind="ExternalInput")
    npk = nc.dram_tensor("npk", [P, S // P, P], dt, kind="ExternalInput")
    fs = nc.dram_tensor("fs", [P, N_SLOTS * 2, P], dt, kind="ExternalInput")
    a2 = nc.dram_tensor("a2", [P, 2, 2, P], dt, kind="ExternalInput")
    # single planar output: row = plane*B + batch (plane 0 = real, 1 = imag)
    # -- matches the psum/staging partition layout, so every store is one
    # full-128-partition DMA instead of two 64-partition ones
    out2 = nc.dram_tensor("out2", [2 * B, N_SLOTS * W], f32,
                          kind="ExternalOutput")

    with tile.TileContext(nc) as tc:
        with (
            tc.tile_pool(name="const", bufs=1) as const,
            tc.tile_pool(name="ltp", bufs=9) as ltp,
            tc.tile_pool(name="psum", bufs=1, space=bass.MemorySpace.PSUM) as psum,
            tc.tile_pool(name="stage", bufs=1) as stage,
        ):
            # npk streams in window-sized pieces as the pairs consume it; the
            # first pieces go on the scalar ring so chunk 0 leads the sync
            # ring and the first matmul starts as early as possible.
            npk_sb = const.tile([P, S // P, P], dt)
            nc.scalar.dma_start(npk_sb[:, 0:8, :], npk.ap()[:, 0:8, :])
            nc.scalar.dma_start(npk_sb[:, 8:16, :], npk.ap()[:, 8:16, :])
            fs_sb = const.tile([P, N_SLOTS * 2, P], dt)
            a2_sb = const.tile([P, 2, 2, P], dt)
            fsi_sb = const.tile([P, N_SLOTS * 2, P], dt)

            ps = [psum.tile([P, W], f32, name=f"acc{j}", tag=f"acc{j}")
                  for j in range(N_SLOTS)]
            st = stage.tile([P, 6, W], f32)
            stB = stage.tile([P, 2, W], f32)
            n_dma = 0
            npk_prefetch = {7: (16, 32), 12: (32, 48), 16: (48, 64)}

            def chunk_dma(n_chunk, ck, flat):
                nonlocal n_dma
                ltc = ltp.tile([P, 16, P], lt_dt, tag="lt", name=f"lt{n_chunk}")
                dma_eng = nc.sync if n_dma % 2 == 0 else nc.scalar
                n_dma += 1
                chunk_inst = dma_eng.dma_start(
                    ltc[:, :ck, :],
                    lt.ap()[flat * P * P:(flat + ck) * P * P].rearrange(
                        "(p n m) -> p n m", p=P, n=ck))
                # fs/a2 aren't needed until the first slots complete at the
                # end of pair 0 -- keep them (and the npk prefetches) behind
                # early chunks with explicit edges so the scheduler can't
                # hoist these dep-free const loads ahead of the chunk stream.
                if n_chunk == 3:
                    fs_inst = nc.sync.dma_start(fs_sb[:], fs.ap())
                    add_dep_helper(fs_inst.ins, chunk_inst.ins, sync=False,
                                   reason="defer fs behind first chunk")
                if n_chunk == 4:
                    a2_inst = dma_eng.dma_start(a2_sb[:], a2.ap())
                    add_dep_helper(a2_inst.ins, chunk_inst.ins, sync=False,
                                   reason="defer a2 behind chunk")
                    # slots complete in ascending order -> derive ascending
                    for g in range(N_SLOTS * 2):
                        nc.vector.tensor_scalar_mul(fsi_sb[:, g, 0:B],
                                                    fs_sb[:, g, B:2 * B], -1.0)
                        nc.vector.tensor_copy(fsi_sb[:, g, B:2 * B],
                                              fs_sb[:, g, 0:B])
                # prefetch the next pair's noise window mid-pair
                if n_chunk in npk_prefetch:
                    lo, hi = npk_prefetch[n_chunk]
                    pf_inst = dma_eng.dma_start(npk_sb[:, lo:hi, :],
                                                npk.ap()[:, lo:hi, :])
                    add_dep_helper(pf_inst.ins, chunk_inst.ins, sync=False,
                                   reason="defer npk prefetch behind chunk")
                return ltc

            def fir_mms(j, stop):
                # FIR: stream A_r against [xr|xi], A_i against [-xi|xr]
                for sdx in (0, 1):
                    for c in (0, 1):
                        g = j * 2 + c
                        src = fs_sb if sdx == 0 else fsi_sb
                        nc.tensor.matmul(ps[j][:], src[:, g, :],
                                         a2_sb[:, sdx, c, :],
                                         start=False,
                                         stop=(stop and sdx == 1 and c == 1))

            def finish_slot(j):
                # slot j's accumulation is complete: evacuate and stream out
                dst = st[:, j, :] if j < 6 else stB[:, j - 6, :]
                nc.vector.tensor_copy(dst, ps[j][:])


            for n_chunk, (j, kt0, ck, flat) in enumerate(CONSUME):
                ltc = chunk_dma(n_chunk, ck, flat)
                # slots 6/7: their FIR only needs fs/a2, so it runs during
                # pair 2, shortening the serial chain after the last chunk
                fir_early = j >= 6 and kt0 == 32
                last_wins_stop = not (j >= 6)
                for i in range(ck):
                    is_last = kt0 + ck == SLOT_KT[j] and i == ck - 1
                    nc.tensor.matmul(ps[j][:], npk_sb[:, kt0 + i, :],
                                     ltc[:, i, :],
                                     start=(kt0 + i == 0),
                                     stop=(is_last and not last_wins_stop))
                if fir_early:
                    fir_mms(j, stop=False)
                if kt0 + ck == SLOT_KT[j]:
                    if last_wins_stop:
                        fir_mms(j, stop=True)
                    finish_slot(j)
            # all stores emitted after the load stream so they never steal
            # DMA-engine time from the chunk loads; the first two fire as
            # soon as their copies land (in the loads' natural gaps)
            nc.sync.dma_start(out2.ap()[:, :4 * W],
                              st[:, 0:4].rearrange("p j w -> p (j w)"))
            nc.scalar.dma_start(out2.ap()[:, 4 * W:6 * W],
                                st[:, 4:6].rearrange("p j w -> p (j w)"))
            nc.scalar.dma_start(out2.ap()[:, 7 * W:], stB[:, 1, :])
            nc.sync.dma_start(out2.ap()[:, 6 * W:7 * W], stB[:, 0, :])

    nc.compile()
    return nc


def _sbuf_image(arr_ktpm):
    """[nkt*128, m] k-tile-major -> SBUF image [128, nkt*m]."""
    nktp, m = arr_ktpm.shape
    nkt = nktp // P
    return np.ascontiguousarray(
        arr_ktpm.reshape(nkt, P, m).transpose(1, 0, 2).reshape(P, nkt * m))


def _prep_inputs(x_real, x_imag, a_real, a_imag, L, noise_r, noise_i, N0,
                 dt_name: str):
    mixed8 = dt_name == "mixed8"
    if mixed8:
        import ml_dtypes
        np_dt = np.float16
        lt_np_dt = ml_dtypes.float8_e3m4
        lt_scale, npk_scale = np.float32(C_LT), np.float32(1.0 / C_LT)
    else:
        np_dt = _DT_NP[dt_name]
        lt_np_dt = np_dt
        lt_scale, npk_scale = np.float32(1.0), np.float32(1.0)

    scale = np.float32(np.sqrt(0.5 * np.power(10.0, np.float64(N0[0]) / 10.0)))

    # packed scaled noise [S, 128]: cols 0:64 real, 64:128 imag
    npk = np.empty((S, 2 * B), np.float32)
    npk[:, :B] = (npk_scale * scale * noise_r).T
    npk[:, B:] = (npk_scale * scale * noise_i).T
    npk = _sbuf_image(npk.astype(np_dt)).reshape(P, S // P, P)

    # x transposed and zero-padded by H on both sides: row r <-> x col r - H
    xpad = np.zeros((S + 2 * H, 2 * B), np.float32)
    xpad[H:H + S, :B] = x_real.T
    xpad[H:H + S, B:] = x_imag.T
    xpad = xpad.astype(np_dt)

    # banded Toeplitz of the taps: A[r, j] = a[j + 2H - r] (valid range only)
    a2 = np.zeros((2, 2 * P, P), np.float32)
    rr = np.arange(2 * P)[:, None]
    jj = np.arange(W)[None, :]
    tap_idx = jj + 2 * H - rr
    valid = (tap_idx >= 0) & (tap_idx < T)
    a2[0][valid] = np.asarray(a_real, np.float32)[tap_idx[valid]]
    a2[1][valid] = np.asarray(a_imag, np.float32)[tap_idx[valid]]
    a2 = _sbuf_image(a2.reshape(2 * 2 * P, P).astype(np_dt)).reshape(P, 2, 2, P)

    L = np.asarray(L, np.float32)

    in_maps = []
    for k in range(N_CORES):
        ltpack = np.zeros((TOT_KT * P * P,), lt_np_dt)
        for j, kt0, ck, flat in CONSUME:
            beta = 8 * j + k
            rows_real = P * (beta + 1)     # non-zero extent in t of strip beta
            r0 = P * kt0                   # this chunk covers t rows r0:r1
            nreal = min(max(rows_real - r0, 0), ck * P)
            if nreal <= 0:
                continue
            block = np.zeros((ck * P, W), lt_np_dt)
            block[:nreal] = np.asarray(
                lt_scale * L[P * beta:P * (beta + 1), r0:r0 + nreal],
                lt_np_dt).T
            img = block.reshape(ck, P, W).transpose(1, 0, 2)
            ltpack[flat * P * P:(flat + ck) * P * P] = img.ravel()

        fsk = np.empty((N_SLOTS * 2, P, 2 * B), np_dt)
        for j in range(N_SLOTS):
            s0 = P * (8 * j + k)           # global first output col of slot
            fsk[j * 2] = xpad[s0:s0 + P]           # [xr | xi] k-tile 0
            fsk[j * 2 + 1] = xpad[s0 + P:s0 + 2 * P]  # k-tile 1
        fsk = _sbuf_image(fsk.reshape(N_SLOTS * 2 * P, 2 * B)).reshape(
            P, N_SLOTS * 2, P)
        in_maps.append({"lt": ltpack, "npk": npk, "fs": fsk, "a2": a2})
    return in_maps


def kernel(x_real, x_imag, a_real, a_imag, L, noise_r, noise_i, N0):
    global LAST_RUN_SECONDS
    inputs = dict(x_real=np.asarray(x_real, np.float32),
                  x_imag=np.asarray(x_imag, np.float32),
                  a_real=np.asarray(a_real, np.float32),
                  a_imag=np.asarray(a_imag, np.float32),
                  L=np.asarray(L, np.float32),
                  noise_r=np.asarray(noise_r, np.float32),
                  noise_i=np.asarray(noise_i, np.float32),
                  N0=np.asarray(N0, np.float32))

    if NOISE_DT not in _CACHE:
        _CACHE[NOISE_DT] = _build_program(NOISE_DT)
    nc = _CACHE[NOISE_DT]

    in_maps = _prep_inputs(**inputs, dt_name=NOISE_DT)

    t0 = time.time()
    res = run_bass_kernel_spmd(nc, in_maps, core_ids=list(range(N_CORES)))
    LAST_RUN_SECONDS = time.time() - t0

    planar = np.empty((2, B, N_SLOTS, N_CORES, W), np.float32)
    for k in range(N_CORES):
        o = res.results[k]["out2"].reshape(2, B, N_SLOTS, W)
        planar[0, :, :, k] = o[0]
        planar[1, :, :, k] = o[1]
    full = np.empty((B, S, 2), np.float32)
    full[:, :, 0] = planar[0].reshape(B, S)
    full[:, :, 1] = planar[1].reshape(B, S)
    return full



# revision 2
# speedup vs baseline: 1.6499x; 1.6499x over previous
"""Additive noise channel kernel for 8 Trainium2 NeuronCores.

Computes out[b, s, 0:2] = complex_FIR(x, a)[b, s] + (L @ (scale * noise))[b, s]
with B=64, S=8192, T=129 taps, L lower-triangular [S, S].

Strategy
--------
The dominant cost is reading L (256 MB fp32, half of it zeros).  We shard the
OUTPUT dim S across the 8 cores so each core reads only its columns of L^T,
and we exploit the triangular structure with a staircase assignment that is
perfectly SPMD-uniform: core k takes the eight 128-column strips
beta = 8j + k (j = 0..7).  Strip slot j is padded to a uniform extent of
8*(j+1) k-tiles of 128 rows (the minimal uniform cover of the triangle for
128-wide strips), so every core runs the identical instruction stream on 288
k-tiles of packed L^T.

Precision / engine use ("dr8" mode): both noise-path operands (L^T and the
raw noise) are fp8e4m3, so the noise matmuls run pairwise in DoubleRow mode
(two 128-row k-tiles contracted per instruction at 0.5 cycles/row -- 4x the
fp16 row rate) and npk is half the bytes of the old fp16 packing.  The
runtime SNR scale is folded into the host-packed L^T (lt = 64*scale*L^T) and
the tap Toeplitz (a2 = 64*a), so one compile-time 1/64 rescale at PSUM
evacuation restores units for both the noise and FIR contributions; the
noise tensor is stored as raw unit-variance e4m3.  x and the taps stay fp16
(the FIR dominates the output scale, so fp8 there would cost ~50x more
output error than fp8 on the noise path).  Accumulation is fp32 in PSUM;
outputs are stored fp16 (threshold is 2e-2, fp16 store adds ~5e-4).

On-device everything is TensorE matmuls accumulating in PSUM:
  * noise coloring: lhsT = [noise_r^T | noise_i^T] (K=2x128 DoubleRow pair)
                    rhs  = L^T tile pair (fp8e4)
    -> psum rows 0:64 = real part, rows 64:128 = imag part; one stream of L
    feeds both real and imag outputs.
  * complex FIR: expressed as x_ext^T @ A where A is the banded Toeplitz
    matrix of the taps, folded into the same PSUM accumulation
    (yr = xr*Ar - xi*Ai, yi = xr*Ai + xi*Ar); the second stationary
    [-xi | xr] is derived on the otherwise-idle VectorE.

Schedule: window-pair-major -- pair p covers k-tiles [16p, 16p+16) of every
still-active slot, so the noise-stationary demand spreads evenly instead of
front-loading; completed slots evacuate + stream out mid-kernel (completing
slots go first within pairs 1-3 so their chains overlap the pair's stream;
pair 0's go last because their FIR needs the late-arriving constants), and
slots 6/7's FIR runs a pair early, so the tail after the last chunk is one
short matmul chain.  The fs/a2/npk constant loads are pinned behind specific
chunks with sync=False dependency edges: without them the Tile scheduler
hoists these dep-free loads ahead of the chunk stream (multi-us PE stall);
anchored too early they displace pair-0 chunk bytes.

All DRAM inputs are packed host-side in SBUF-image layout (partition-major,
1-2 KB contiguous runs per partition, chunk sequence in exact consumption
order) so the HBM read stream is sequential and every DMA descriptor is
>=1 KB.  Outputs are written planar (real / imag) and interleaved on the
host via one merged planar tensor (row = plane*B + batch, matching the psum
partition layout, so each store is a single full-128-partition DMA).
"""

import os
import sys
import time

for _p in ("/opt/trn_rl_repo", "/root/.axon_site/_ro/trn_rl_repo"):
    if _p not in sys.path:
        sys.path.append(_p)

# the bass kernel executes through jax/PJRT on the axon-tunneled NeuronCores
os.environ.setdefault("JAX_PLATFORMS", "axon,cpu")

import numpy as np

import concourse.bass as bass
import concourse.mybir as mybir
import concourse.tile as tile
from concourse.tile import add_dep_helper
from concourse import bacc
from concourse.bass_utils import run_bass_kernel_spmd

B = 64          # batch
S = 8192        # block size
T = 129         # taps
H = (T - 1) // 2  # 64
P = 128         # partitions / k-tile
N_CORES = 8
N_SLOTS = 8     # strips per core
W = 128         # strip width (output columns per slot)
SLOT_KT = [8 * (j + 1) for j in range(N_SLOTS)]   # padded k-tiles per slot
TOT_KT = sum(SLOT_KT)  # 288

# Window-pair-major schedule: pair p covers k-tiles [16p, 16p+16).  All slots
# still alive advance through that window together, so the npk (noise) demand
# spreads evenly across the kernel instead of front-loading, and slots 2p /
# 2p+1 finish in pair p (their outputs stream out mid-kernel).
# CONSUME entries: (slot j, first k-tile kt0, n k-tiles ck, flat offset);
# chunks are laid out back-to-back in DRAM in this (consumption) order.
CONSUME = []
_flat = 0
for _p in range(4):
    # pairs 1-3: completing slots FIRST -- their chunks arrive earliest in
    # the pair, so their FIR + psum evacuation + store all overlap the rest
    # of the pair's chunk stream instead of gating the kernel tail.  Pair 0
    # keeps them LAST: slots 0/1's FIR needs the fs/a2/fsi constants, which
    # only land a few us in.
    if _p == 0:
        _order = list(range(2, N_SLOTS)) + [0, 1]
    elif _p == 3:
        # slot 7 last, with its final chunk split so the chain after the
        # very last byte is only a few matmuls + one store
        _order = [6, 7]
    else:
        _order = [2 * _p, 2 * _p + 1] + list(range(2 * _p + 2, N_SLOTS))
    for _j in _order:
        _ck = 8 if _j == 2 * _p else 16
        CONSUME.append((_j, 16 * _p, _ck, _flat))
        _flat += _ck
assert _flat == TOT_KT
_j9, _kt9, _ck9, _fl9 = CONSUME[-1]
CONSUME[-1:] = [(_j9, _kt9, 12, _fl9), (_j9, _kt9 + 12, 4, _fl9 + 12)]

# Precision mode.
# "dr8":   L^T and noise in fp8e4m3 (DoubleRow-paired noise matmuls, 0.5
#          cycles/row), x/taps fp16, fp32 PSUM, fp16 stores. ~4e-3 rel err.
# "mixed8": L^T in fp8e3m4 (pre-scaled by C_LT, folded back via the fp16
#          noise stationary), everything else fp16. ~1.3e-3.
# "float16": all operands fp16 (~3e-4).  "float32": exact (~2e-7), slow.
NOISE_DT = "dr8"

C_LT = 64.0  # fp8 pre-scale; dr8 folds 64*scale into lt and 64 into a2

_DT_NP = {"float32": np.float32, "float16": np.float16}

# mode -> (lt dt, npk dt, fs/a2 dt, out dt, doublerow)
_MODES = {
    "dr8": ("float8e4", "float8e4", "float16", "float16", True),
    "mixed8": ("float8e3", "float16", "float16", "float32", False),
    "float16": ("float16", "float16", "float16", "float32", False),
    "float32": ("float32", "float32", "float32", "float32", False),
}

LAST_RUN_SECONDS = None
_CACHE = {}


def _build_program(dt_name: str):
    lt_dtn, npk_dtn, op_dtn, out_dtn, dr = _MODES[dt_name]
    lt_dt = getattr(mybir.dt, lt_dtn)
    npk_dt = getattr(mybir.dt, npk_dtn)
    dt = getattr(mybir.dt, op_dtn)
    out_dt = getattr(mybir.dt, out_dtn)
    f32 = mybir.dt.float32
    dr_mode = mybir.MatmulPerfMode.DoubleRow if dr else None

    nc = bacc.Bacc("TRN2", target_bir_lowering=False, debug=False,
                   num_devices=N_CORES)

    # all inputs are SBUF images: [128 partitions, free...]; lt is a flat
    # sequence of per-chunk SBUF images in consumption order
    lt = nc.dram_tensor("lt", [TOT_KT * P * P], lt_dt, kind="ExternalInput")
    npk = nc.dram_tensor("npk", [P, S // P, P], npk_dt, kind="ExternalInput")
    fs = nc.dram_tensor("fs", [P, N_SLOTS * 2, P], dt, kind="ExternalInput")
    a2 = nc.dram_tensor("a2", [P, 2, 2, P], dt, kind="ExternalInput")
    # single planar output: row = plane*B + batch (plane 0 = real, 1 = imag)
    # -- matches the psum/staging partition layout, so every store is one
    # full-128-partition DMA instead of two 64-partition ones
    out2 = nc.dram_tensor("out2", [2 * B, N_SLOTS * W], out_dt,
                          kind="ExternalOutput")

    with tile.TileContext(nc) as tc:
        with (
            tc.tile_pool(name="const", bufs=1) as const,
            tc.tile_pool(name="ltp", bufs=9) as ltp,
            tc.tile_pool(name="psum", bufs=1, space=bass.MemorySpace.PSUM) as psum,
            tc.tile_pool(name="stage", bufs=1) as stage,
        ):
            # npk streams in window-sized pieces as the pairs consume it; the
            # first pieces go on the scalar ring so chunk 0 leads the sync
            # ring and the first matmul starts as early as possible.
            npk_sb = const.tile([P, S // P, P], npk_dt)
            nc.scalar.dma_start(npk_sb[:, 0:8, :], npk.ap()[:, 0:8, :])
            nc.scalar.dma_start(npk_sb[:, 8:16, :], npk.ap()[:, 8:16, :])
            fs_sb = const.tile([P, N_SLOTS * 2, P], dt)
            a2_sb = const.tile([P, 2, 2, P], dt)
            fsi_sb = const.tile([P, N_SLOTS * 2, P], dt)

            ps = [psum.tile([P, W], f32, name=f"acc{j}", tag=f"acc{j}")
                  for j in range(N_SLOTS)]
            st = stage.tile([P, 6, W], out_dt)
            stB = stage.tile([P, 2, W], out_dt)
            n_dma = 0
            npk_prefetch = {7: (16, 32), 12: (32, 48), 16: (48, 64)}

            def chunk_dma(n_chunk, ck, flat):
                nonlocal n_dma
                ltc = ltp.tile([P, 16, P], lt_dt, tag="lt", name=f"lt{n_chunk}")
                dma_eng = nc.sync if n_dma % 2 == 0 else nc.scalar
                n_dma += 1
                chunk_inst = dma_eng.dma_start(
                    ltc[:, :ck, :],
                    lt.ap()[flat * P * P:(flat + ck) * P * P].rearrange(
                        "(p n m) -> p n m", p=P, n=ck))
                # fs/a2 aren't needed until the first slots complete at the
                # end of pair 0 -- keep them (and the npk prefetches) behind
                # early chunks with explicit edges so the scheduler can't
                # hoist these dep-free const loads ahead of the chunk stream.
                if n_chunk == 3:
                    fs_inst = nc.sync.dma_start(fs_sb[:], fs.ap())
                    add_dep_helper(fs_inst.ins, chunk_inst.ins, sync=False,
                                   reason="defer fs behind first chunk")
                if n_chunk == 4:
                    a2_inst = dma_eng.dma_start(a2_sb[:], a2.ap())
                    add_dep_helper(a2_inst.ins, chunk_inst.ins, sync=False,
                                   reason="defer a2 behind chunk")
                    # slots complete in ascending order -> derive ascending
                    for g in range(N_SLOTS * 2):
                        nc.vector.tensor_scalar_mul(fsi_sb[:, g, 0:B],
                                                    fs_sb[:, g, B:2 * B], -1.0)
                        nc.vector.tensor_copy(fsi_sb[:, g, B:2 * B],
                                              fs_sb[:, g, 0:B])
                # prefetch the next pair's noise window mid-pair
                if n_chunk in npk_prefetch:
                    lo, hi = npk_prefetch[n_chunk]
                    pf_inst = dma_eng.dma_start(npk_sb[:, lo:hi, :],
                                                npk.ap()[:, lo:hi, :])
                    add_dep_helper(pf_inst.ins, chunk_inst.ins, sync=False,
                                   reason="defer npk prefetch behind chunk")
                return ltc

            def fir_mms(j, stop):
                # FIR: stream A_r against [xr|xi], A_i against [-xi|xr]
                for sdx in (0, 1):
                    for c in (0, 1):
                        g = j * 2 + c
                        src = fs_sb if sdx == 0 else fsi_sb
                        nc.tensor.matmul(ps[j][:], src[:, g, :],
                                         a2_sb[:, sdx, c, :],
                                         start=False,
                                         stop=(stop and sdx == 1 and c == 1))

            def finish_slot(j):
                # slot j's accumulation is complete: evacuate and stream out
                dst = st[:, j, :] if j < 6 else stB[:, j - 6, :]
                if dr:
                    # undo the folded C_LT pre-scale (exact power of two)
                    nc.vector.tensor_scalar_mul(dst, ps[j][:], 1.0 / C_LT)
                else:
                    nc.vector.tensor_copy(dst, ps[j][:])


            for n_chunk, (j, kt0, ck, flat) in enumerate(CONSUME):
                ltc = chunk_dma(n_chunk, ck, flat)
                # slots 6/7: their FIR only needs fs/a2, so it runs during
                # pair 2, shortening the serial chain after the last chunk
                fir_early = j >= 6 and kt0 == 32
                last_wins_stop = not (j >= 6)
                if dr:
                    for u in range(ck // 2):
                        i = 2 * u
                        is_last = kt0 + ck == SLOT_KT[j] and i == ck - 2
                        nc.tensor.matmul(
                            ps[j][:],
                            npk_sb[:, kt0 + i:kt0 + i + 2, :],
                            ltc[:, i:i + 2, :],
                            start=(kt0 + i == 0),
                            stop=(is_last and not last_wins_stop),
                            perf_mode=dr_mode)
                else:
                    for i in range(ck):
                        is_last = kt0 + ck == SLOT_KT[j] and i == ck - 1
                        nc.tensor.matmul(ps[j][:], npk_sb[:, kt0 + i, :],
                                         ltc[:, i, :],
                                         start=(kt0 + i == 0),
                                         stop=(is_last and not last_wins_stop))
                if fir_early:
                    fir_mms(j, stop=False)
                if kt0 + ck == SLOT_KT[j]:
                    if last_wins_stop:
                        fir_mms(j, stop=True)
                    finish_slot(j)
            # all stores emitted after the load stream so they never steal
            # DMA-engine time from the chunk loads; the first two fire as
            # soon as their copies land (in the loads' natural gaps)
            nc.sync.dma_start(out2.ap()[:, :4 * W],
                              st[:, 0:4].rearrange("p j w -> p (j w)"))
            nc.scalar.dma_start(out2.ap()[:, 4 * W:6 * W],
                                st[:, 4:6].rearrange("p j w -> p (j w)"))
            nc.scalar.dma_start(out2.ap()[:, 7 * W:], stB[:, 1, :])
            nc.sync.dma_start(out2.ap()[:, 6 * W:7 * W], stB[:, 0, :])

    nc.compile()
    return nc


def _sbuf_image(arr_ktpm):
    """[nkt*128, m] k-tile-major -> SBUF image [128, nkt*m]."""
    nktp, m = arr_ktpm.shape
    nkt = nktp // P
    return np.ascontiguousarray(
        arr_ktpm.reshape(nkt, P, m).transpose(1, 0, 2).reshape(P, nkt * m))


def _prep_inputs(x_real, x_imag, a_real, a_imag, L, noise_r, noise_i, N0,
                 dt_name: str):
    import ml_dtypes
    lt_dtn, npk_dtn, op_dtn, _, dr = _MODES[dt_name]
    _np_of = {"float32": np.float32, "float16": np.float16,
              "float8e3": ml_dtypes.float8_e3m4,
              "float8e4": ml_dtypes.float8_e4m3}
    np_dt = _np_of[op_dtn]
    npk_np_dt = _np_of[npk_dtn]
    lt_np_dt = _np_of[lt_dtn]

    scale = np.float32(np.sqrt(0.5 * np.power(10.0, np.float64(N0[0]) / 10.0)))

    if dr:
        # lt = (C_LT*scale)*L^T, npk = raw noise, a2 = C_LT*a; one 1/C_LT
        # rescale at evacuation restores units on both paths.
        lt_scale = np.float32(C_LT) * scale
        npk_scale = np.float32(1.0)
        npk_pre = np.float32(1.0)
        a2_scale = np.float32(C_LT)
    elif dt_name == "mixed8":
        lt_scale = np.float32(C_LT)
        npk_scale = np.float32(1.0 / C_LT)
        npk_pre = scale
        a2_scale = np.float32(1.0)
    else:
        lt_scale = np.float32(1.0)
        npk_scale = np.float32(1.0)
        npk_pre = scale
        a2_scale = np.float32(1.0)

    # packed noise [S, 128]: cols 0:64 real, 64:128 imag
    npkf = np.empty((S, 2 * B), np.float32)
    npkf[:, :B] = (npk_scale * npk_pre * noise_r).T
    npkf[:, B:] = (npk_scale * npk_pre * noise_i).T
    npk = _sbuf_image(npkf.astype(npk_np_dt)).reshape(P, S // P, P)

    # x transposed and zero-padded by H on both sides: row r <-> x col r - H
    xpad = np.zeros((S + 2 * H, 2 * B), np.float32)
    xpad[H:H + S, :B] = x_real.T
    xpad[H:H + S, B:] = x_imag.T
    xpad = xpad.astype(np_dt)

    # banded Toeplitz of the taps: A[r, j] = a[j + 2H - r] (valid range only)
    a2 = np.zeros((2, 2 * P, P), np.float32)
    rr = np.arange(2 * P)[:, None]
    jj = np.arange(W)[None, :]
    tap_idx = jj + 2 * H - rr
    valid = (tap_idx >= 0) & (tap_idx < T)
    a2[0][valid] = a2_scale * np.asarray(a_real, np.float32)[tap_idx[valid]]
    a2[1][valid] = a2_scale * np.asarray(a_imag, np.float32)[tap_idx[valid]]
    a2 = _sbuf_image(a2.reshape(2 * 2 * P, P).astype(np_dt)).reshape(P, 2, 2, P)

    L = np.asarray(L, np.float32)

    in_maps = []
    for k in range(N_CORES):
        ltpack = np.zeros((TOT_KT * P * P,), lt_np_dt)
        for j, kt0, ck, flat in CONSUME:
            beta = 8 * j + k
            rows_real = P * (beta + 1)     # non-zero extent in t of strip beta
            r0 = P * kt0                   # this chunk covers t rows r0:r1
            nreal = min(max(rows_real - r0, 0), ck * P)
            if nreal <= 0:
                continue
            block = np.zeros((ck * P, W), lt_np_dt)
            block[:nreal] = np.asarray(
                lt_scale * L[P * beta:P * (beta + 1), r0:r0 + nreal],
                lt_np_dt).T
            img = block.reshape(ck, P, W).transpose(1, 0, 2)
            ltpack[flat * P * P:(flat + ck) * P * P] = img.ravel()

        fsk = np.empty((N_SLOTS * 2, P, 2 * B), np_dt)
        for j in range(N_SLOTS):
            s0 = P * (8 * j + k)           # global first output col of slot
            fsk[j * 2] = xpad[s0:s0 + P]           # [xr | xi] k-tile 0
            fsk[j * 2 + 1] = xpad[s0 + P:s0 + 2 * P]  # k-tile 1
        fsk = _sbuf_image(fsk.reshape(N_SLOTS * 2 * P, 2 * B)).reshape(
            P, N_SLOTS * 2, P)
        in_maps.append({"lt": ltpack, "npk": npk, "fs": fsk, "a2": a2})
    return in_maps


def kernel(x_real, x_imag, a_real, a_imag, L, noise_r, noise_i, N0):
    global LAST_RUN_SECONDS
    inputs = dict(x_real=np.asarray(x_real, np.float32),
                  x_imag=np.asarray(x_imag, np.float32),
                  a_real=np.asarray(a_real, np.float32),
                  a_imag=np.asarray(a_imag, np.float32),
                  L=np.asarray(L, np.float32),
                  noise_r=np.asarray(noise_r, np.float32),
                  noise_i=np.asarray(noise_i, np.float32),
                  N0=np.asarray(N0, np.float32))

    if NOISE_DT not in _CACHE:
        _CACHE[NOISE_DT] = _build_program(NOISE_DT)
    nc = _CACHE[NOISE_DT]

    in_maps = _prep_inputs(**inputs, dt_name=NOISE_DT)

    t0 = time.time()
    res = run_bass_kernel_spmd(nc, in_maps, core_ids=list(range(N_CORES)))
    LAST_RUN_SECONDS = time.time() - t0

    planar = np.empty((2, B, N_SLOTS, N_CORES, W), np.float32)
    for k in range(N_CORES):
        o = np.asarray(res.results[k]["out2"],
                       np.float32).reshape(2, B, N_SLOTS, W)
        planar[0, :, :, k] = o[0]
        planar[1, :, :, k] = o[1]
    full = np.empty((B, S, 2), np.float32)
    full[:, :, 0] = planar[0].reshape(B, S)
    full[:, :, 1] = planar[1].reshape(B, S)
    return full


# revision 6
# speedup vs baseline: 1.6884x; 1.0233x over previous
"""Additive noise channel kernel for 8 Trainium2 NeuronCores.

Computes out[b, s, 0:2] = complex_FIR(x, a)[b, s] + (L @ (scale * noise))[b, s]
with B=64, S=8192, T=129 taps, L lower-triangular [S, S].

Strategy ("fine8" mode)
-----------------------
The dominant cost is reading L (256 MB fp32, half zeros), so the kernel is
DMA-bound: every byte of L^T read is time on the (serialized, ~360 GB/s)
DMA device.  The output columns are sharded across cores in 16-column fine
strips (strip sigma covers columns [16*sigma, 16*sigma+16), core k owns
sigma == k mod 8).  With this interleave, core k's m-th strip always ends in
k-tile m, so the per-slot k-extents (m+1 tiles) are identical on every core:
the staircase of the triangular L packs with ZERO padding bytes -- 2080
128x16 tiles = 4.26 MB/core in fp8 (vs 4.72 MB for the 128-wide-strip
staircase, whose SPMD-uniform cover needs 28 padding k-tiles).

Noise k-tile t multiplies exactly the strips m >= t, which are contiguous
psum columns [16t, 1024) -- so each k-tile needs just one or two matmuls
(split at the 512-col psum bank boundary).  Both noise operands are fp8e4m3
and adjacent k-tiles are contracted pairwise in DoubleRow mode (0.5
cycles/row); the 16 leading columns of the even tile are covered by a tiny
single matmul, which also makes the pair's two tiles align exactly -- no
zero-padding blocks.  The runtime SNR scale folds into the host-packed L^T
(lt = 64*scale*L^T) and tap Toeplitz (a2 = 64*a), so one compile-time 1/64
rescale at PSUM evacuation restores units.

The FIR keeps the COARSE 128-column-strip sharding (its x-window locality
breaks under fine interleave: fine strips would need ~4x the x bytes), so
its column set differs from the noise shard's.  The two parts are therefore
stored separately -- FIR fp16, noise fp8e4m3 (noise is a small additive
component of the output, so fp8 storage costs ~2e-3 relative error) -- and
summed on the host during unsharding, which is free.

Schedule: the lt stream is 10 large chunks (fewer DMA instructions keeps
the shared descriptor-gen device off the critical path); noise psum columns
finalize monotonically left-to-right, so evacuation trickles behind the
chunk stream.  The FIR product and both its stores plus the first noise
store are complete mid-stream and anchored behind the last chunk: they fill
the DMA device while the final chunk's sem-prop -> matmul -> evac -> store
chain (the unavoidable ~2.5 us tail) plays out.  The last chunk is a single
48-byte-per-partition pair so almost no compute rides on the final bytes.
"""

import os
import sys
import time

for _p in ("/opt/trn_rl_repo", "/root/.axon_site/_ro/trn_rl_repo"):
    if _p not in sys.path:
        sys.path.append(_p)

# the bass kernel executes through jax/PJRT on the axon-tunneled NeuronCores
os.environ.setdefault("JAX_PLATFORMS", "axon,cpu")

import numpy as np

import concourse.bass as bass
import concourse.mybir as mybir
import concourse.tile as tile
from concourse.tile import add_dep_helper
from concourse import bacc
from concourse.bass_utils import run_bass_kernel_spmd

B = 64          # batch
S = 8192        # block size
T = 129         # taps
H = (T - 1) // 2  # 64
P = 128         # partitions / k-tile
N_CORES = 8
N_SLOTS = 8     # coarse strips per core (FIR sharding)
W = 128         # coarse strip width
NKT = S // P    # 64 noise k-tiles
NPAIR = NKT // 2
FINE = 16       # fine strip width (noise sharding)
NFS = S // FINE // N_CORES   # 64 fine strips per core
COLS = NFS * FINE            # 1024 psum/output columns per core

C_LT = 64.0  # fp8 pre-scale; folded into lt (64*scale*L) and a2 (64*a)

# --- fine8 noise layout -----------------------------------------------------
# noise k-tile t feeds psum cols [16t, 1024): width w(t) = 1024 - 16t.
# DoubleRow pair p = tiles (2p, 2p+1): a 16-wide single matmul covers tile
# 2p's leading cols [32p, 32p+16); the DR matmul covers [32p+16, 1024) with
# i=0 -> tile 2p cols 16.., i=1 -> tile 2p+1 (exact alignment, no padding).
# Packed pair block per partition: [single16 | tile2p[16:] | tile2p+1] =
# 2*w(2p) - 16 bytes.


def _wp(p):
    return COLS - 32 * p


PAIR_BYTES = [2 * _wp(p) - 16 for p in range(NPAIR)]
LT_BYTES = sum(PAIR_BYTES)  # 33280 per partition

# lt DMA chunks: groups of pairs.  Large chunks keep the descriptor-gen
# device cold; the last chunk is a single tiny pair so the tail chain after
# the final bytes is minimal.
CHUNKS = [[0, 1], [2, 3], [4, 5], [6, 7], [8, 9, 10], [11, 12, 13],
          [14, 15, 16, 17], [18, 19, 20, 21, 22],
          [23, 24, 25, 26, 27, 28, 29, 30], [31]]
CHUNK_BYTES = [sum(PAIR_BYTES[p] for p in ch) for ch in CHUNKS]
CHUNK_OFF = np.cumsum([0] + CHUNK_BYTES).tolist()
MAX_CHUNK = max(CHUNK_BYTES)
# psum cols finalized after each chunk: 32 * (last pair + 1)
CHUNK_DONE = [32 * (ch[-1] + 1) for ch in CHUNKS]

NOISE_DT = "fine8"

_MODES = {"fine8": None}

LAST_RUN_SECONDS = None
_CACHE = {}


def _build_program(dt_name: str):
    assert dt_name == "fine8"
    f32 = mybir.dt.float32
    f16 = mybir.dt.float16
    f8 = mybir.dt.float8e4
    DR = mybir.MatmulPerfMode.DoubleRow

    nc = bacc.Bacc("TRN2", target_bir_lowering=False, debug=False,
                   num_devices=N_CORES)

    lt = nc.dram_tensor("lt", [P * LT_BYTES], f8, kind="ExternalInput")
    npk = nc.dram_tensor("npk", [P, NKT, P], f8, kind="ExternalInput")
    fs = nc.dram_tensor("fs", [P, N_SLOTS * 2, P], f16, kind="ExternalInput")
    a2 = nc.dram_tensor("a2", [P, 2, 2, P], f16, kind="ExternalInput")
    # separate outputs: FIR on the coarse shard (fp16), noise on the fine
    # shard (fp8); host sums them during unsharding.  row = plane*B + batch.
    fir_out = nc.dram_tensor("fir", [2 * B, N_SLOTS * W], f16,
                             kind="ExternalOutput")
    noise_out = nc.dram_tensor("noise", [2 * B, COLS], f8,
                               kind="ExternalOutput")

    with tile.TileContext(nc) as tc:
        with (
            tc.tile_pool(name="const", bufs=1) as const,
            tc.tile_pool(name="ltp", bufs=4) as ltp,
            tc.tile_pool(name="psum", bufs=1, space=bass.MemorySpace.PSUM) as psum,
            tc.tile_pool(name="stage", bufs=1) as stage,
        ):
            npk_sb = const.tile([P, NKT, P], f8)
            # first noise window leads the stream so pair-0 can start early
            nc.sync.dma_start(npk_sb[:, 0:16, :], npk.ap()[:, 0:16, :])
            fs_sb = const.tile([P, N_SLOTS * 2, P], f16)
            a2_sb = const.tile([P, 2, 2, P], f16)
            fsi_sb = const.tile([P, N_SLOTS * 2, P], f16)

            psA = psum.tile([P, 512], f32, name="npsA", tag="npsA")
            psB = psum.tile([P, 512], f32, name="npsB", tag="npsB")
            psF = [psum.tile([P, 512], f32, name=f"fps{i}", tag=f"fps{i}")
                   for i in range(2)]
            n_st = stage.tile([P, COLS], f8)
            f_st = stage.tile([P, N_SLOTS * W], f16)

            def seg_mm(lo, hi, lhsT, rhs3, perf, start, stops):
                """matmul into noise psum cols [lo, hi), split at bank 512.
                rhs3 is indexed in region-relative cols (caller aligns);
                stops = (stop for bank A segment, stop for bank B segment)."""
                for s0, s1, ps, stop in ((lo, min(hi, 512), psA, stops[0]),
                                         (max(lo, 512), hi, psB, stops[1])):
                    if s0 >= s1:
                        continue
                    r0, r1 = s0 - lo, s1 - lo
                    rhs = rhs3[:, :, r0:r1] if perf else rhs3[:, r0:r1]
                    nc.tensor.matmul(ps[:, s0 % 512:(s1 - 1) % 512 + 1],
                                     lhsT, rhs, start=start, stop=stop,
                                     perf_mode=perf, skip_group_check=True)

            def fir_mms(j):
                for sdx in (0, 1):
                    for c in (0, 1):
                        g = j * 2 + c
                        src = fs_sb if sdx == 0 else fsi_sb
                        nc.tensor.matmul(
                            psF[j // 4][:, 128 * (j % 4):128 * (j % 4) + 128],
                            src[:, g, :], a2_sb[:, sdx, c, :],
                            start=(sdx == 0 and c == 0),
                            stop=(sdx == 1 and c == 1),
                            skip_group_check=True)

            n_dma = 0
            done = 0
            for ci, pairs in enumerate(CHUNKS):
                cb = CHUNK_BYTES[ci]
                ltc = ltp.tile([P, MAX_CHUNK], f8, tag="lt", name=f"lt{ci}")
                dma_eng = nc.sync if n_dma % 2 == 0 else nc.scalar
                n_dma += 1
                chunk_inst = dma_eng.dma_start(
                    ltc[:, :cb],
                    lt.ap()[CHUNK_OFF[ci] * P:CHUNK_OFF[ci + 1] * P].rearrange(
                        "(p w) -> p w", p=P))
                # anchor dep-free const loads behind the chunk stream so the
                # scheduler can't hoist them ahead of the lt bytes
                if ci == 1:
                    fs_inst = nc.sync.dma_start(fs_sb[:], fs.ap())
                    add_dep_helper(fs_inst.ins, chunk_inst.ins, sync=False,
                                   reason="defer fs")
                if ci == 2:
                    a2_inst = dma_eng.dma_start(a2_sb[:], a2.ap())
                    add_dep_helper(a2_inst.ins, chunk_inst.ins, sync=False,
                                   reason="defer a2")
                    np2 = nc.sync.dma_start(npk_sb[:, 16:40, :],
                                            npk.ap()[:, 16:40, :])
                    add_dep_helper(np2.ins, chunk_inst.ins, sync=False,
                                   reason="defer npk2")
                    for g in range(N_SLOTS * 2):
                        nc.vector.tensor_scalar_mul(fsi_sb[:, g, 0:B],
                                                    fs_sb[:, g, B:2 * B], -1.0)
                        nc.vector.tensor_copy(fsi_sb[:, g, B:2 * B],
                                              fs_sb[:, g, 0:B])
                if ci == 5:
                    np3 = nc.scalar.dma_start(npk_sb[:, 40:64, :],
                                              npk.ap()[:, 40:64, :])
                    add_dep_helper(np3.ins, chunk_inst.ins, sync=False,
                                   reason="defer npk3")

                off = 0
                for p in pairs:
                    w = _wp(p)
                    # single: tile 2p leading 16 cols -> [32p, 32p+16)
                    seg_mm(32 * p, 32 * p + 16, npk_sb[:, 2 * p, :],
                           ltc[:, off:off + 16], None, start=(p == 0),
                           stops=(False, False))
                    # DoubleRow: tiles (2p, 2p+1) -> [32p+16, 1024)
                    dr = ltc[:, off + 16:off + 16 + 2 * (w - 16)].rearrange(
                        "q (two w) -> q two w", two=2)
                    # bank A's last writer is pair 15's A segment; bank B's
                    # is pair 31's
                    seg_mm(32 * p + 16, COLS, npk_sb[:, 2 * p:2 * p + 2, :],
                           dr, DR, start=(p == 0),
                           stops=(p == 15, p == NPAIR - 1))
                    off += PAIR_BYTES[p]

                if ci == 3:
                    for j in range(N_SLOTS):
                        fir_mms(j)
                    for i in range(2):
                        nc.vector.tensor_scalar_mul(
                            f_st[:, 512 * i:512 * (i + 1)], psF[i][:],
                            1.0 / C_LT)

                # evacuate the noise psum cols this chunk finalized
                new_done = CHUNK_DONE[ci]
                for s0, s1, ps in ((done, min(new_done, 512), psA),
                                   (max(done, 512), new_done, psB)):
                    if s0 < s1:
                        nc.vector.tensor_scalar_mul(
                            n_st[:, s0:s1], ps[:, s0 % 512:(s1 - 1) % 512 + 1],
                            1.0 / C_LT)
                done = new_done

                if ci == len(CHUNKS) - 1:
                    # tail fillers: ready long ago, anchored behind the last
                    # chunk so they occupy the DMA device during the final
                    # sem->matmul->evac chain
                    st1 = nc.scalar.dma_start(fir_out.ap(), f_st[:])
                    add_dep_helper(st1.ins, chunk_inst.ins, sync=False,
                                   reason="tail filler fir")
                    st2 = nc.scalar.dma_start(noise_out.ap()[:, 0:512],
                                              n_st[:, 0:512])
                    add_dep_helper(st2.ins, chunk_inst.ins, sync=False,
                                   reason="tail filler noiseA")
            # final store: waits on the last evacuation
            nc.sync.dma_start(noise_out.ap()[:, 512:COLS], n_st[:, 512:COLS])

    nc.compile()
    return nc


def _sbuf_image(arr_ktpm):
    """[nkt*128, m] k-tile-major -> SBUF image [128, nkt*m]."""
    nktp, m = arr_ktpm.shape
    nkt = nktp // P
    return np.ascontiguousarray(
        arr_ktpm.reshape(nkt, P, m).transpose(1, 0, 2).reshape(P, nkt * m))


def _prep_inputs(x_real, x_imag, a_real, a_imag, L, noise_r, noise_i, N0,
                 dt_name: str):
    import ml_dtypes
    f8 = ml_dtypes.float8_e4m3

    scale = np.float32(np.sqrt(0.5 * np.power(10.0, np.float64(N0[0]) / 10.0)))
    lt_scale = np.float32(C_LT) * scale

    # packed raw noise [S, 128]: cols 0:64 real, 64:128 imag (e4m3)
    npkf = np.empty((S, 2 * B), np.float32)
    npkf[:, :B] = noise_r.T
    npkf[:, B:] = noise_i.T
    npk = _sbuf_image(npkf.astype(f8)).reshape(P, NKT, P)

    # x transposed, zero-padded by H: row r <-> x col r - H
    xpad = np.zeros((S + 2 * H, 2 * B), np.float32)
    xpad[H:H + S, :B] = x_real.T
    xpad[H:H + S, B:] = x_imag.T
    xpad = xpad.astype(np.float16)

    # banded Toeplitz of the taps, pre-scaled by C_LT
    a2 = np.zeros((2, 2 * P, P), np.float32)
    rr = np.arange(2 * P)[:, None]
    jj = np.arange(W)[None, :]
    tap_idx = jj + 2 * H - rr
    valid = (tap_idx >= 0) & (tap_idx < T)
    a2[0][valid] = C_LT * np.asarray(a_real, np.float32)[tap_idx[valid]]
    a2[1][valid] = C_LT * np.asarray(a_imag, np.float32)[tap_idx[valid]]
    a2 = _sbuf_image(a2.reshape(2 * 2 * P, P).astype(np.float16)).reshape(
        P, 2, 2, P)

    L = np.asarray(L, np.float32)

    in_maps = []
    for k in range(N_CORES):
        # fine-strip L^T stream: tile t = L^T[128t:128t+128, cols of strips
        # m >= t], strips m -> global cols 128m+16k+[0,16)
        tiles = []
        for t in range(NKT):
            cols = (128 * np.arange(t, NFS)[:, None] + 16 * k
                    + np.arange(FINE)[None, :]).ravel()
            blk = (lt_scale * L[cols, 128 * t:128 * (t + 1)].T).astype(f8)
            tiles.append(np.ascontiguousarray(blk))   # [128, 16*(64-t)]
        stream = np.empty((P, LT_BYTES), f8)
        off = 0
        for p in range(NPAIR):
            w = _wp(p)
            stream[:, off:off + 16] = tiles[2 * p][:, :16]
            stream[:, off + 16:off + w] = tiles[2 * p][:, 16:]
            stream[:, off + w:off + 2 * w - 16] = tiles[2 * p + 1]
            off += 2 * w - 16
        assert off == LT_BYTES
        # flatten chunk-by-chunk so each chunk is contiguous in DRAM
        ltpack = np.concatenate(
            [stream[:, CHUNK_OFF[ci]:CHUNK_OFF[ci + 1]].ravel()
             for ci in range(len(CHUNKS))])

        # coarse-strip x windows for the FIR (identical to the 128-col shard)
        fsk = np.empty((N_SLOTS * 2, P, 2 * B), np.float16)
        for j in range(N_SLOTS):
            s0 = P * (8 * j + k)
            fsk[j * 2] = xpad[s0:s0 + P]
            fsk[j * 2 + 1] = xpad[s0 + P:s0 + 2 * P]
        fsk = _sbuf_image(fsk.reshape(N_SLOTS * 2 * P, 2 * B)).reshape(
            P, N_SLOTS * 2, P)
        in_maps.append({"lt": ltpack, "npk": npk, "fs": fsk, "a2": a2})
    return in_maps


def kernel(x_real, x_imag, a_real, a_imag, L, noise_r, noise_i, N0):
    global LAST_RUN_SECONDS
    inputs = dict(x_real=np.asarray(x_real, np.float32),
                  x_imag=np.asarray(x_imag, np.float32),
                  a_real=np.asarray(a_real, np.float32),
                  a_imag=np.asarray(a_imag, np.float32),
                  L=np.asarray(L, np.float32),
                  noise_r=np.asarray(noise_r, np.float32),
                  noise_i=np.asarray(noise_i, np.float32),
                  N0=np.asarray(N0, np.float32))

    if NOISE_DT not in _CACHE:
        _CACHE[NOISE_DT] = _build_program(NOISE_DT)
    nc = _CACHE[NOISE_DT]

    in_maps = _prep_inputs(**inputs, dt_name=NOISE_DT)

    t0 = time.time()
    res = run_bass_kernel_spmd(nc, in_maps, core_ids=list(range(N_CORES)))
    LAST_RUN_SECONDS = time.time() - t0

    full = np.zeros((2, B, S), np.float32)
    for k in range(N_CORES):
        fir = np.asarray(res.results[k]["fir"],
                         np.float32).reshape(2, B, N_SLOTS, W)
        # coarse: slot j -> cols [128*(8j+k), +128)
        fir_view = full.reshape(2, B, N_SLOTS, N_CORES, W)
        fir_view[:, :, :, k, :] += fir
        noi = np.asarray(res.results[k]["noise"],
                         np.float32).reshape(2, B, NFS, FINE)
        # fine: strip m -> cols 128m + 16k + [0,16)
        noi_view = full.reshape(2, B, NFS, N_CORES, FINE)
        noi_view[:, :, :, k, :] += noi
    out = np.empty((B, S, 2), np.float32)
    out[:, :, 0] = full[0]
    out[:, :, 1] = full[1]
    return out


# revision 37
# speedup vs baseline: 1.7942x; 1.0626x over previous
"""Additive noise channel kernel for 8 Trainium2 NeuronCores.

Computes out[b, s, 0:2] = complex_FIR(x, a)[b, s] + (L @ (scale * noise))[b, s]
with B=64, S=8192, T=129 taps, L lower-triangular [S, S].

Strategy ("fine8" mode)
-----------------------
The dominant cost is reading L (256 MB fp32, half zeros), so the kernel is
DMA-bound: every byte of L^T read is time on the (serialized, ~360 GB/s)
DMA device.  The output columns are sharded across cores in 16-column fine
strips (strip sigma covers columns [16*sigma, 16*sigma+16), core k owns
sigma == k mod 8).  With this interleave, core k's m-th strip always ends in
k-tile m, so the per-slot k-extents (m+1 tiles) are identical on every core:
the staircase of the triangular L packs with ZERO padding bytes -- 2080
128x16 tiles = 4.26 MB/core in fp8 (vs 4.72 MB for the 128-wide-strip
staircase, whose SPMD-uniform cover needs 28 padding k-tiles).

Noise k-tile t multiplies exactly the strips m >= t, which are contiguous
psum columns [16t, 1024) -- so each k-tile needs just one or two matmuls
(split at the 512-col psum bank boundary).  Both noise operands are fp8e4m3
and adjacent k-tiles are contracted pairwise in DoubleRow mode (0.5
cycles/row); the 16 leading columns of the even tile are covered by a tiny
single matmul, which also makes the pair's two tiles align exactly -- no
zero-padding blocks.  The runtime SNR scale folds into the host-packed L^T
(lt = 64*scale*L^T) and tap Toeplitz (a2 = 64*a), so one compile-time 1/64
rescale at PSUM evacuation restores units.

The FIR keeps the COARSE 128-column-strip sharding (its x-window locality
breaks under fine interleave: fine strips would need ~4x the x bytes), so
its column set differs from the noise shard's.  The two parts are therefore
stored separately -- FIR fp16, noise fp8e4m3 (noise is a small additive
component of the output, so fp8 storage costs ~2e-3 relative error) -- and
summed on the host during unsharding, which is free.

Schedule: the lt stream is 10 large chunks (fewer DMA instructions keeps
the shared descriptor-gen device off the critical path); noise psum columns
finalize monotonically left-to-right, so evacuation trickles behind the
chunk stream.  The FIR product and both its stores plus the first noise
store are complete mid-stream and anchored behind the last chunk: they fill
the DMA device while the final chunk's sem-prop -> matmul -> evac -> store
chain (the unavoidable ~2.5 us tail) plays out.  The last chunk is a single
48-byte-per-partition pair so almost no compute rides on the final bytes.
"""

import os
import sys
import time

for _p in ("/opt/trn_rl_repo", "/root/.axon_site/_ro/trn_rl_repo"):
    if _p not in sys.path:
        sys.path.append(_p)

# the bass kernel executes through jax/PJRT on the axon-tunneled NeuronCores
os.environ.setdefault("JAX_PLATFORMS", "axon,cpu")

import numpy as np

import concourse.bass as bass
import concourse.mybir as mybir
import concourse.tile as tile
from concourse.tile import add_dep_helper
from concourse import bacc
from concourse.bass_utils import run_bass_kernel_spmd

B = 64          # batch
S = 8192        # block size
T = 129         # taps
H = (T - 1) // 2  # 64
P = 128         # partitions / k-tile
N_CORES = 8
N_SLOTS = 8     # coarse strips per core (FIR sharding)
W = 128         # coarse strip width
NKT = S // P    # 64 noise k-tiles
NPAIR = NKT // 2
FINE = 16       # fine strip width (noise sharding)
NFS = S // FINE // N_CORES   # 64 fine strips per core
COLS = NFS * FINE            # 1024 psum/output columns per core

C_LT = 64.0  # fp8 pre-scale; folded into lt (64*scale*L) and a2 (64*a)

# --- fine8 noise layout -----------------------------------------------------
# noise k-tile t feeds psum cols [16t, 1024): width w(t) = 1024 - 16t.
# DoubleRow pair p = tiles (2p, 2p+1): a 16-wide single matmul covers tile
# 2p's leading cols [32p, 32p+16); the DR matmul covers [32p+16, 1024) with
# i=0 -> tile 2p cols 16.., i=1 -> tile 2p+1 (exact alignment, no padding).
# Packed pair block per partition: [single16 | tile2p[16:] | tile2p+1] =
# 2*w(2p) - 16 bytes.


def _wp(p):
    return COLS - 32 * p


PAIR_BYTES = [2 * _wp(p) - 16 for p in range(NPAIR)]
LT_BYTES = sum(PAIR_BYTES)  # 33280 per partition

# lt DMA chunks: groups of pairs.  Large chunks keep the descriptor-gen
# device cold; the last chunk is a single tiny pair so the tail chain after
# the final bytes is minimal.
CHUNKS = [[0, 1], [2, 3], [4, 5], [6, 7], [8, 9, 10], [11, 12, 13],
          [14, 15, 16, 17], [18, 19, 20, 21, 22],
          [23, 24, 25, 26, 27], [28, 29, 30, 31]]
CHUNK_BYTES = [sum(PAIR_BYTES[p] for p in ch) for ch in CHUNKS]
CHUNK_OFF = np.cumsum([0] + CHUNK_BYTES).tolist()
MAX_CHUNK = max(CHUNK_BYTES)
# psum cols finalized after each chunk: 32 * (last pair + 1)
CHUNK_DONE = [32 * (ch[-1] + 1) for ch in CHUNKS]

NOISE_DT = "fine8"

_MODES = {"fine8": None}

LAST_RUN_SECONDS = None
_CACHE = {}


def _build_program(dt_name: str):
    assert dt_name == "fine8"
    f32 = mybir.dt.float32
    f16 = mybir.dt.float16
    f8 = mybir.dt.float8e4
    f8x = mybir.dt.float8e3
    DR = mybir.MatmulPerfMode.DoubleRow

    nc = bacc.Bacc("TRN2", target_bir_lowering=False, debug=False,
                   num_devices=N_CORES)

    i16 = mybir.dt.int16
    lt = nc.dram_tensor("lt", [P * LT_BYTES], f8, kind="ExternalInput")
    npk = nc.dram_tensor("npk", [P, NKT, P], f8, kind="ExternalInput")
    fs = nc.dram_tensor("fs", [P, N_SLOTS * 2, P], f8x, kind="ExternalInput")
    a2 = nc.dram_tensor("a2", [P, 2, 2, P], f16, kind="ExternalInput")
    # separate outputs: FIR on the coarse shard (fp16), noise on the fine
    # shard (fp8); host sums them during unsharding.  row = plane*B + batch.
    fir_out = nc.dram_tensor("fir", [2 * B, N_SLOTS * W], f16,
                             kind="ExternalOutput")
    noise_out = nc.dram_tensor("noise", [2 * B, COLS], f8,
                               kind="ExternalOutput")

    with tile.TileContext(nc) as tc:
        with (
            tc.tile_pool(name="const", bufs=1) as const,
            tc.tile_pool(name="ltp", bufs=4) as ltp,
            tc.tile_pool(name="psum", bufs=1, space=bass.MemorySpace.PSUM) as psum,
            tc.tile_pool(name="stage", bufs=1) as stage,
        ):
            npk_sb = const.tile([P, NKT, P], f8)
            # first noise window leads the stream so pair-0 can start early
            nc.sync.dma_start(npk_sb[:, 0:16, :], npk.ap()[:, 0:16, :])
            fs_sb = const.tile([P, N_SLOTS * 2, P], f8x)
            a2_sb = const.tile([P, 2, 2, P], f16)
            fsi_sb = const.tile([P, N_SLOTS * 2, P], f16)

            # noise psum in THREE tiles cut at WAR boundaries: psA's last
            # writer is pair 15, psB1's pair 27, psB2's pair 31.  Evacuating
            # a tile after its last writer never blocks later matmuls (the
            # dependency tracker is tile-granular, so a read of a shared
            # tile would stall every later write to it).
            psA = psum.tile([P, 512], f32, name="npsA", tag="npsA")
            psB1 = psum.tile([P, 384], f32, name="npsB1", tag="npsB1")
            psB2 = psum.tile([P, 128], f32, name="npsB2", tag="npsB2")
            psF = [psum.tile([P, 512], f32, name=f"fps{i}", tag=f"fps{i}")
                   for i in range(2)]
            n_st = stage.tile([P, COLS], f8)
            f_st = stage.tile([P, N_SLOTS * W], f16)

            last_mm = [None]
            SEGS = ((0, 512, psA), (512, 896, psB1), (896, COLS, psB2))

            def seg_mm(lo, hi, lhsT, rhs3, perf, start, stops):
                """matmul into noise psum cols [lo, hi), split at the psum
                tile boundaries.  rhs3 is indexed in region-relative cols;
                stops = per-segment stop flags."""
                for (b0, b1, ps), stop in zip(SEGS, stops):
                    s0, s1 = max(lo, b0), min(hi, b1)
                    if s0 >= s1:
                        continue
                    r0, r1 = s0 - lo, s1 - lo
                    rhs = rhs3[:, :, r0:r1] if perf else rhs3[:, r0:r1]
                    last_mm[0] = nc.tensor.matmul(
                        ps[:, s0 - b0:s1 - b0],
                        lhsT, rhs, start=start, stop=stop,
                        perf_mode=perf, skip_group_check=True)

            def fir_mms(j):
                for sdx in (0, 1):
                    for c in (0, 1):
                        g = j * 2 + c
                        src = fs_sb if sdx == 0 else fsi_sb
                        nc.tensor.matmul(
                            psF[j // 4][:, 128 * (j % 4):128 * (j % 4) + 128],
                            src[:, g, :], a2_sb[:, sdx, c, :],
                            start=(sdx == 0 and c == 0),
                            stop=(sdx == 1 and c == 1),
                            skip_group_check=True)

            n_dma = 0
            done = 0
            for ci, pairs in enumerate(CHUNKS):
                cb = CHUNK_BYTES[ci]
                ltc = ltp.tile([P, MAX_CHUNK], f8, tag="lt", name=f"lt{ci}")
                dma_eng = nc.sync if n_dma % 2 == 0 else nc.scalar
                n_dma += 1
                chunk_inst = dma_eng.dma_start(
                    ltc[:, :cb],
                    lt.ap()[CHUNK_OFF[ci] * P:CHUNK_OFF[ci + 1] * P].rearrange(
                        "(p w) -> p w", p=P))
                # anchor dep-free const loads behind the chunk stream so the
                # scheduler can't hoist them ahead of the lt bytes
                if ci == 1:
                    fs_inst = nc.sync.dma_start(fs_sb[:], fs.ap())
                    add_dep_helper(fs_inst.ins, chunk_inst.ins, sync=False,
                                   reason="defer fs")
                if ci == 2:
                    a2_inst = dma_eng.dma_start(a2_sb[:], a2.ap())
                    add_dep_helper(a2_inst.ins, chunk_inst.ins, sync=False,
                                   reason="defer a2")
                    np2 = nc.sync.dma_start(npk_sb[:, 16:40, :],
                                            npk.ap()[:, 16:40, :])
                    add_dep_helper(np2.ins, chunk_inst.ins, sync=False,
                                   reason="defer npk2")
                    for g in range(N_SLOTS * 2):
                        nc.vector.tensor_scalar_mul(fsi_sb[:, g, 0:B],
                                                    fs_sb[:, g, B:2 * B], -1.0)
                        nc.vector.tensor_copy(fsi_sb[:, g, B:2 * B],
                                              fs_sb[:, g, 0:B])
                if ci == 5:
                    np3 = nc.scalar.dma_start(npk_sb[:, 40:64, :],
                                              npk.ap()[:, 40:64, :])
                    add_dep_helper(np3.ins, chunk_inst.ins, sync=False,
                                   reason="defer npk3")

                off = 0
                for p in pairs:
                    w = _wp(p)
                    # single: tile 2p leading 16 cols -> [32p, 32p+16)
                    seg_mm(32 * p, 32 * p + 16, npk_sb[:, 2 * p, :],
                           ltc[:, off:off + 16], None, start=(p == 0),
                           stops=(False, False, False))
                    # DoubleRow: tiles (2p, 2p+1) -> [32p+16, 1024)
                    dr = ltc[:, off + 16:off + 16 + 2 * (w - 16)].rearrange(
                        "q (two w) -> q two w", two=2)
                    seg_mm(32 * p + 16, COLS, npk_sb[:, 2 * p:2 * p + 2, :],
                           dr, DR, start=(p == 0),
                           stops=(p == 15, p == 27, p == NPAIR - 1))
                    off += PAIR_BYTES[p]

                if ci == 3:
                    for j in range(N_SLOTS):
                        fir_mms(j)
                    for i in range(2):
                        nc.vector.tensor_scalar_mul(
                            f_st[:, 512 * i:512 * (i + 1)], psF[i][:],
                            1.0 / C_LT)

                # evacuate each noise psum tile once, right after its LAST
                # writer's chunk: psA after pair 15 (chunk 6), psB1 after
                # pair 27 (chunk 8), psB2 after pair 31 (last chunk).  This
                # gives three PE-drain points that land between chunk
                # ladders, and no evac ever blocks a later matmul.
                # mid-stream evacs ride the Activation engine so the DVE is
                # free the moment the final evac's gate opens
                if ci == 6:
                    nc.scalar.activation(n_st[:, 0:512], psA[:],
                                         mybir.ActivationFunctionType.Copy,
                                         scale=1.0 / C_LT)
                if ci == 8:
                    nc.scalar.activation(n_st[:, 512:896], psB1[:],
                                         mybir.ActivationFunctionType.Copy,
                                         scale=1.0 / C_LT)
                if ci == len(CHUNKS) - 1:
                    # tail fillers on the last chunk's queue (cannot overtake
                    # it on the DMA device), keeping the device busy during
                    # the final sem -> matmul -> evac -> trigger chain
                    st1 = dma_eng.dma_start(fir_out.ap(), f_st[:])
                    add_dep_helper(st1.ins, chunk_inst.ins, sync=False,
                                   reason="tail filler fir")
                    st2 = dma_eng.dma_start(noise_out.ap()[:, 0:512],
                                            n_st[:, 0:512])
                    add_dep_helper(st2.ins, chunk_inst.ins, sync=False,
                                   reason="tail filler noiseA")
                    ev1 = nc.vector.tensor_scalar_mul(
                        n_st[:, 896:COLS], psB2[:], 1.0 / C_LT)
                    add_dep_helper(ev1.ins, last_mm[0].ins, sync=True,
                                   reason="final evac after all matmuls")


            nc.sync.dma_start(noise_out.ap()[:, 512:COLS],
                              n_st[:, 512:COLS])

    nc.compile()
    return nc


def _sbuf_image(arr_ktpm):
    """[nkt*128, m] k-tile-major -> SBUF image [128, nkt*m]."""
    nktp, m = arr_ktpm.shape
    nkt = nktp // P
    return np.ascontiguousarray(
        arr_ktpm.reshape(nkt, P, m).transpose(1, 0, 2).reshape(P, nkt * m))


def _prep_inputs(x_real, x_imag, a_real, a_imag, L, noise_r, noise_i, N0,
                 dt_name: str):
    import ml_dtypes
    f8 = ml_dtypes.float8_e4m3

    scale = np.float32(np.sqrt(0.5 * np.power(10.0, np.float64(N0[0]) / 10.0)))
    lt_scale = np.float32(C_LT) * scale

    # packed raw noise [S, 128]: cols 0:64 real, 64:128 imag (e4m3)
    npkf = np.empty((S, 2 * B), np.float32)
    npkf[:, :B] = noise_r.T
    npkf[:, B:] = noise_i.T
    npk = _sbuf_image(npkf.astype(f8)).reshape(P, NKT, P)

    # x transposed, zero-padded by H: row r <-> x col r - H
    xpad = np.zeros((S + 2 * H, 2 * B), np.float32)
    xpad[H:H + S, :B] = x_real.T
    xpad[H:H + S, B:] = x_imag.T
    xpad = xpad.astype(ml_dtypes.float8_e3m4)

    # banded Toeplitz of the taps, pre-scaled by C_LT
    a2 = np.zeros((2, 2 * P, P), np.float32)
    rr = np.arange(2 * P)[:, None]
    jj = np.arange(W)[None, :]
    tap_idx = jj + 2 * H - rr
    valid = (tap_idx >= 0) & (tap_idx < T)
    a2[0][valid] = C_LT * np.asarray(a_real, np.float32)[tap_idx[valid]]
    a2[1][valid] = C_LT * np.asarray(a_imag, np.float32)[tap_idx[valid]]
    a2 = _sbuf_image(a2.reshape(2 * 2 * P, P).astype(np.float16)).reshape(
        P, 2, 2, P)

    L = np.asarray(L, np.float32)

    in_maps = []
    for k in range(N_CORES):
        # fine-strip L^T stream: tile t = L^T[128t:128t+128, cols of strips
        # m >= t], strips m -> global cols 128m+16k+[0,16)
        tiles = []
        for t in range(NKT):
            cols = (128 * np.arange(t, NFS)[:, None] + 16 * k
                    + np.arange(FINE)[None, :]).ravel()
            blk = (lt_scale * L[cols, 128 * t:128 * (t + 1)].T).astype(f8)
            tiles.append(np.ascontiguousarray(blk))   # [128, 16*(64-t)]
        stream = np.empty((P, LT_BYTES), f8)
        off = 0
        for p in range(NPAIR):
            w = _wp(p)
            stream[:, off:off + 16] = tiles[2 * p][:, :16]
            stream[:, off + 16:off + w] = tiles[2 * p][:, 16:]
            stream[:, off + w:off + 2 * w - 16] = tiles[2 * p + 1]
            off += 2 * w - 16
        assert off == LT_BYTES
        # flatten chunk-by-chunk so each chunk is contiguous in DRAM
        ltpack = np.concatenate(
            [stream[:, CHUNK_OFF[ci]:CHUNK_OFF[ci + 1]].ravel()
             for ci in range(len(CHUNKS))])

        # coarse-strip x windows for the FIR (identical to the 128-col shard)
        fsk = np.empty((N_SLOTS * 2, P, 2 * B), ml_dtypes.float8_e3m4)
        for j in range(N_SLOTS):
            s0 = P * (8 * j + k)
            fsk[j * 2] = xpad[s0:s0 + P]
            fsk[j * 2 + 1] = xpad[s0 + P:s0 + 2 * P]
        fsk = _sbuf_image(fsk.reshape(N_SLOTS * 2 * P, 2 * B)).reshape(
            P, N_SLOTS * 2, P)
        in_maps.append({"lt": ltpack, "npk": npk, "fs": fsk, "a2": a2})
    return in_maps


def kernel(x_real, x_imag, a_real, a_imag, L, noise_r, noise_i, N0):
    global LAST_RUN_SECONDS
    inputs = dict(x_real=np.asarray(x_real, np.float32),
                  x_imag=np.asarray(x_imag, np.float32),
                  a_real=np.asarray(a_real, np.float32),
                  a_imag=np.asarray(a_imag, np.float32),
                  L=np.asarray(L, np.float32),
                  noise_r=np.asarray(noise_r, np.float32),
                  noise_i=np.asarray(noise_i, np.float32),
                  N0=np.asarray(N0, np.float32))

    if NOISE_DT not in _CACHE:
        _CACHE[NOISE_DT] = _build_program(NOISE_DT)
    nc = _CACHE[NOISE_DT]

    in_maps = _prep_inputs(**inputs, dt_name=NOISE_DT)

    t0 = time.time()
    res = run_bass_kernel_spmd(nc, in_maps, core_ids=list(range(N_CORES)))
    LAST_RUN_SECONDS = time.time() - t0

    full = np.zeros((2, B, S), np.float32)
    for k in range(N_CORES):
        fir = np.asarray(res.results[k]["fir"],
                         np.float32).reshape(2, B, N_SLOTS, W)
        # coarse: slot j -> cols [128*(8j+k), +128)
        fir_view = full.reshape(2, B, N_SLOTS, N_CORES, W)
        fir_view[:, :, :, k, :] += fir
        noi = np.asarray(res.results[k]["noise"],
                         np.float32).reshape(2, B, NFS, FINE)
        # fine: strip m -> cols 128m + 16k + [0,16)
        noi_view = full.reshape(2, B, NFS, N_CORES, FINE)
        noi_view[:, :, :, k, :] += noi
    out = np.empty((B, S, 2), np.float32)
    out[:, :, 0] = full[0]
    out[:, :, 1] = full[1]
    return out
